# revision 49
# baseline (speedup 1.0000x reference)
"""ALiBi flash attention (B=2, S=2048, E=1024, H=16, D=64) on 8 TRN2 NeuronCores.

Sharding: data parallel over batch (2) x tensor parallel over heads (16 -> 4
head-slots per core, heads interleaved h = g + 4k so every core sees the same
ALiBi band structure slot-by-slot and one SPMD program serves all cores).

Per core: project q/k/v for its 4 heads (256 channels), run banded causal
attention per head with the ALiBi bias folded into the QK^T matmul as extra
contraction rows (slope*j and -slope*i, bf16-split 3 ways so the reduced
mantissa cannot hurt the bias), then the output projection against its 256
columns of Wo. Host sums the 4 partial y's per batch and adds bo + Wo @ bv
(the v bias commutes through softmax-weighted averaging).

Layout/scheduling tricks vs the original version:
- bf16 matmul wire format (tolerance allows it); y partials stored as bf16.
- odd head-slots keep their q/k data on SBUF partitions 64:128 (aug rows at
  58:64, zeros below) so projection evictions never need a partition-shift
  staging DMA; their QK matmul contracts the full 0:128 partition range.
- softmax normalize: reshape the PV ones-row to 64 lanes (contiguous-run
  DMAs only), DVE reciprocal, tiny DRAM bounce to broadcast across
  partitions, DVE multiply. No 4-byte-descriptor scatter transposes.
- the final slot streams: each 512-col query group is normalized as soon as
  its last key tile lands, and the output projection + y store of the
  previous group interleave into the attention loop; projection quarters 2-3
  interleave into slot-0's key loop. All [PT,512] f32 PSUM accumulators
  (projection/QK/out-proj) share one 4-deep ring, outacc keeps 4 banks.
- DMA issues that wait on compute never ride the Activation queue (they
  would head-block the softmax EXPs); they go to sync/gpsimd instead.
"""

import math
import os

import numpy as np

import concourse.bacc as bacc
import concourse.mybir as mybir
from concourse.bass_utils import run_bass_kernel_spmd
from concourse.tile import TileContext

B, S, E, H, D = 2, 2048, 1024, 16, 64
NCORES, SLOTS = 8, 4
CG = SLOTS * D          # channels per core (256)
PT = 128                # partition tile
NT = S // PT            # 16 sequence tiles
KAUG = D + 6            # contraction rows: 64 data + 3 (slope*j) + 3 (-slope*i)
_BDEF = "1,3,12,16"
BANDS = [int(b) for b in os.environ.get("BANDS", _BDEF).split(",")]
F32 = mybir.dt.float32
BF16 = mybir.dt.bfloat16
F32R = mybir.dt.float32r
MM_DT = {"f32r": F32R, "bf16": BF16}[os.environ.get("MM_DT", "bf16")]
AX = mybir.ActivationFunctionType
OP = mybir.AluOpType

_CACHE = {}


def _alibi_slopes(n):
    def pow2(m):
        start = 2.0 ** (-(2.0 ** (-(math.log2(m) - 3))))
        return [start * (start ** i) for i in range(m)]
    if math.log2(n).is_integer():
        return np.array(pow2(n), dtype=np.float64)
    closest = 2 ** math.floor(math.log2(n))
    extra = pow2(2 * closest)[closest:]
    return np.array(pow2(closest) + extra[: n - closest], dtype=np.float64)


def _round_bf16(x):
    u = np.ascontiguousarray(x, dtype=np.float32).view(np.uint32)
    r = (u + 0x7FFF + ((u >> 16) & 1)) & 0xFFFF0000
    return r.astype(np.uint32).view(np.float32)


def _split3(v):
    hi = _round_bf16(v)
    r1 = (v - hi).astype(np.float32)
    mid = _round_bf16(r1)
    lo = _round_bf16((r1 - mid).astype(np.float32))
    return hi, mid, lo


def _qk_pieces(width):
    return [(a, min(a + 512, width)) for a in range(0, width, 512)]


def _pv_pieces(tj, wb):
    """Global-column pieces for the PV matmuls of key-tile tj.

    Each piece must sit in one 512-col PSUM bank of the accumulator, stay on
    one side of the start-region boundary (columns first written by this tj),
    and not straddle a 1024-col P^T tile edge.
    """
    i_lo = tj * PT
    i_hi = min(tj + wb + 1, NT) * PT
    if tj == 0:
        nr = (i_lo, i_hi)
    else:
        nt_new = tj + wb
        nr = (nt_new * PT, nt_new * PT + PT) if nt_new < NT else None
    bounds = {i_lo, i_hi}
    bounds |= {b for b in range(0, S + 1, 512) if i_lo < b < i_hi}
    bounds |= {i_lo + 512 * t for t in range(1, 16) if i_lo < i_lo + 512 * t < i_hi}
    if nr:
        bounds |= {x for x in nr if i_lo <= x <= i_hi}
    bounds = sorted(bounds)
    pieces = []
    for a, b2 in zip(bounds[:-1], bounds[1:]):
        is_new = nr is not None and nr[0] <= a < nr[1]
        is_diag = a >= tj * PT and b2 <= (tj + 1) * PT
        pieces.append((a, b2, is_new, is_diag))
    return pieces


def _build_program():
    nc = bacc.Bacc(target_bir_lowering=False)
    xT = nc.declare_dram_parameter("xT", [E, S], MM_DT, isOutput=False)
    wqT = nc.declare_dram_parameter("wqT", [E, CG], MM_DT, isOutput=False)
    wkT = nc.declare_dram_parameter("wkT", [E, CG], MM_DT, isOutput=False)
    wvT = nc.declare_dram_parameter("wvT", [E, CG], MM_DT, isOutput=False)
    woT = nc.declare_dram_parameter("woT", [CG, E], MM_DT, isOutput=False)
    bqk = nc.declare_dram_parameter("bqk", [2, CG], F32, isOutput=False)
    aug = nc.declare_dram_parameter("aug", [SLOTS, 12, S], MM_DT, isOutput=False)
    trineg = nc.declare_dram_parameter("trineg", [PT, PT], MM_DT, isOutput=False)
    y = nc.declare_dram_parameter("y", [S, E], MM_DT, isOutput=True)

    with TileContext(nc) as tc, tc.tile_pool(name="pers", bufs=1) as pers:
        # ---- persistent SBUF tensors ----
        wq_sb = pers.tile([PT, 8, CG], MM_DT, name="wq_sb")
        wk_sb = pers.tile([PT, 8, CG], MM_DT, name="wk_sb")
        wv_sb = pers.tile([PT, 8, CG], MM_DT, name="wv_sb")
        wo_sb = pers.tile([PT, 2, E], MM_DT, name="wo_sb")
        bias_sb = pers.tile([PT, 2, 2], F32, name="bias_sb")
        tri_sb = pers.tile([PT, PT], MM_DT, name="tri_sb")
        zbias = pers.tile([PT, 1], F32, name="zbias")
        # even slots: data on partitions 0:64, aug rows 64:70 ([KAUG, S] tile).
        # odd slots: [PT, S] tile — zeros 0:58, aug rows 58:64, data 64:128 so
        # the projection eviction lands without a partition shift; the QK
        # matmul contracts the full 0:128 range (zero rows contribute nothing).
        qa = [pers.tile([KAUG if s % 2 == 0 else PT, S], MM_DT, name=f"qa{s}")
              for s in range(SLOTS)]
        ka = [pers.tile([KAUG if s % 2 == 0 else PT, S], MM_DT, name=f"ka{s}")
              for s in range(SLOTS)]
        v_all = pers.tile([PT, NT, SLOTS, D + 1], MM_DT, name="v_all")
        onorm = pers.tile([PT, 2, S], MM_DT, name="onorm")

        # wq first on gpsimd (kt=0 chunk alone so the first matmul starts
        # after 64KB) and the x quarter-0 chunks first on sync/scalar;
        # constants and aug rows queue behind them.
        wqr = wqT[:, :].rearrange("(t p) c -> p t c", p=PT)
        nc.gpsimd.dma_start(out=wq_sb[:, 0, :], in_=wqr[:, 0, :])
        nc.gpsimd.dma_start(out=wq_sb[:, 1, :], in_=wqr[:, 1, :])
        nc.gpsimd.dma_start(out=wq_sb[:, 2:8, :], in_=wqr[:, 2:8, :])
        nc.vector.memset(zbias, -44.0)
        nc.vector.memset(v_all[:, :, :, D], 1.0)
        for s in range(1, SLOTS, 2):
            nc.vector.memset(ka[s][0:D - 6, :], 0.0)
            nc.vector.memset(qa[s][0:D - 6, :], 0.0)

        # ---- projections + attention in one scope: every [PT,512] f32 PSUM
        # accumulator (projection ps, attention qt, out-proj yp) shares one
        # ring tag, so quarter-3 projections interleave into slot-0 attention.
        QTB = int(os.environ.get("QTB", "4"))
        with tc.tile_pool(name="xp", bufs=3) as xp, \
             tc.tile_pool(name="qkp", bufs=2, space="PSUM") as qkp, \
             tc.tile_pool(name="oap", bufs=1, space="PSUM") as oap, \
             tc.tile_pool(name="ptp", bufs=3) as ptp, \
             tc.tile_pool(name="nrm", bufs=4) as nrm, \
             tc.tile_pool(name="ysb", bufs=3) as ysb, \
             tc.tile_pool(name="drp", bufs=2, space="DRAM") as drp:

            def proj_qk_chunk(xq, ssl, dst, w_sb, scale, brow, ct):
                ps = qkp.tile([PT, 512], F32, tag="qk", bufs=QTB)
                for kt in range(8):
                    nc.tensor.matmul(
                        ps[:, :],
                        w_sb[:, kt, ct * PT:(ct + 1) * PT],
                        xq[:, kt, :],
                        start=(kt == 0), stop=(kt == 7),
                    )
                nc.vector.tensor_scalar(
                    out=dst[2 * ct][0:D, ssl], in0=ps[0:D, :],
                    scalar1=scale, scalar2=bias_sb[0:D, brow, ct:ct + 1],
                    op0=OP.mult, op1=OP.add,
                )
                nc.vector.tensor_scalar(
                    out=dst[2 * ct + 1][D:PT, ssl], in0=ps[D:PT, :],
                    scalar1=scale, scalar2=bias_sb[D:PT, brow, ct:ct + 1],
                    op0=OP.mult, op1=OP.add,
                )

            def proj_v_pair(xq, mtp):
                # two v tiles share one [PT,512] PSUM accumulator; start only
                # on the bank's first write, stop only on its last.
                vps = qkp.tile([PT, 512], F32, tag="qk", bufs=QTB)
                for h in range(2):
                    for kt in range(8):
                        nc.tensor.matmul(
                            vps[:, h * CG:(h + 1) * CG],
                            xq[:, kt, ((mtp + h) % 4) * PT:((mtp + h) % 4 + 1) * PT],
                            wv_sb[:, kt, :],
                            start=(h == 0 and kt == 0), stop=(h == 1 and kt == 7),
                            skip_group_check=True,
                        )
                nc.vector.tensor_scalar(
                    out=v_all[:, mtp:mtp + 2, :, 0:D],
                    in0=vps[:, :].rearrange("p (m a d) -> p m a d", m=2, d=D),
                    scalar1=1.0, scalar2=0.0, op0=OP.mult, op1=OP.add,
                )

            pending = []
            for qt_i in range(4):
                ssl = slice(qt_i * 512, qt_i * 512 + 512)
                xq = xp.tile([PT, 8, 512], MM_DT, tag="xq")
                if qt_i == 0:
                    # split by contraction chunk so the kt=0 matmul can start
                    # as soon as the first 128 rows land; spread the issue
                    # across the sync and scalar sequencers (gpsimd is busy
                    # with wq).
                    for kt8 in range(8):
                        eng = (nc.sync, nc.scalar)[kt8 % 2]
                        eng.dma_start(
                            out=xq[:, kt8, :],
                            in_=xT[:, :].rearrange("(t p) s -> p t s", p=PT)[:, kt8, ssl],
                        )
                    # constants + aug rows, behind the ramp-critical loads
                    nc.sync.dma_start(
                        out=bias_sb, in_=bqk[:, :].rearrange("r (t p) -> p r t", p=PT))
                    nc.sync.dma_start(out=tri_sb, in_=trineg[:, :])
                    for s in range(SLOTS):
                        lo = D if s % 2 == 0 else D - 6
                        nc.sync.dma_start(out=ka[s][lo:lo + 6, :], in_=aug[s, 0:6, :])
                        nc.sync.dma_start(out=qa[s][lo:lo + 6, :], in_=aug[s, 6:12, :])
                    nc.gpsimd.dma_start(
                        out=wk_sb, in_=wkT[:, :].rearrange("(t p) c -> p t c", p=PT))
                    nc.gpsimd.dma_start(
                        out=wv_sb, in_=wvT[:, :].rearrange("(t p) c -> p t c", p=PT))
                else:
                    nc.gpsimd.dma_start(
                        out=xq,
                        in_=xT[:, :].rearrange("(t p) s -> p t s", p=PT)[:, :, ssl],
                    )
                if qt_i == 1:
                    nc.gpsimd.dma_start(
                        out=wo_sb, in_=woT[:, :].rearrange("(t p) e -> p t e", p=PT))
                work = []
                for dst, w_sb, scale, brow in ((qa, wq_sb, 0.125, 0), (ka, wk_sb, 1.0, 1)):
                    for ct in range(2):
                        work.append((proj_qk_chunk, (xq, ssl, dst, w_sb, scale, brow, ct)))
                for mtp in (qt_i * 4, qt_i * 4 + 2):
                    work.append((proj_v_pair, (xq, mtp)))
                if qt_i < 1:
                    for fn, args in work:
                        fn(*args)
                else:
                    # quarters 1-3 interleave into slot-0 attention: two
                    # chunks per key tile keeps every projection eviction
                    # ahead of its first band-1 reader (quarter q's stationary
                    # columns are first read at key tile 4q).
                    pending.extend(work)

            # ---- banded causal attention, one head slot at a time ----

            def out_proj(mt):
                """y rows for query tile mt: yp = onorm[:, :, blk]^T @ wo."""
                blk = slice(mt * PT, (mt + 1) * PT)
                ys = ysb.tile([PT, E], MM_DT, tag="ys")
                for ec in range(2):
                    yp = qkp.tile([PT, 512], F32, tag="qk", bufs=QTB)
                    for ctp in range(2):
                        nc.tensor.matmul(
                            yp[:, :],
                            onorm[:, ctp, blk],
                            wo_sb[:, ctp, ec * 512:(ec + 1) * 512],
                            start=(ctp == 0), stop=(ctp == 1), skip_group_check=True,
                        )
                    esl = slice(ec * 512, (ec + 1) * 512)
                    if (mt + ec) % 2 == 0:
                        nc.scalar.activation(out=ys[:, esl], in_=yp, func=AX.Copy)
                    else:
                        nc.vector.tensor_scalar(out=ys[:, esl], in0=yp, scalar1=1.0,
                                                scalar2=0.0, op0=OP.mult, op1=OP.add)
                q_eng = (nc.sync, nc.gpsimd)[mt % 2]
                q_eng.dma_start(out=y[blk, :], in_=ys)

            for s in range(SLOTS):
                wb = BANDS[s]
                odd = s % 2
                ct = s // 2
                final = s == SLOTS - 1
                ka_ap = ka[s][0:KAUG] if not odd else ka[s][0:PT]
                qa_ap = qa[s][0:KAUG] if not odd else qa[s][0:PT]
                outacc = oap.tile([D + 1, S], F32, tag="outacc")
                # start=True clears has_written for the whole PSUM bank, so it
                # may only be issued on the FIRST matmul touching each 512-col
                # bank of the accumulator (everything later accumulates, with
                # never-written elements overwritten via the cleared bit).
                bank_started = set()
                bank_last = {}
                for tj in range(NT):
                    for (a, b2, _n, _d) in _pv_pieces(tj, wb):
                        bank_last[a // 512] = (tj, a)
                for tj in range(NT):
                    i_lo = tj * PT
                    i_hi = min(tj + wb + 1, NT) * PT
                    width = i_hi - i_lo
                    pvp = _pv_pieces(tj, wb)
                    for T in range((width + 511) // 512):
                        w_t = min(512, width - 512 * T)
                        qt = qkp.tile([PT, 512], F32, tag="qk", bufs=QTB)
                        for (a, b2) in _qk_pieces(w_t):
                            nc.tensor.matmul(
                                qt[:, a:b2],
                                ka_ap[:, i_lo:i_lo + PT],
                                qa_ap[:, i_lo + 512 * T + a:i_lo + 512 * T + b2],
                                start=True, stop=True, skip_group_check=True,
                            )
                        pt_t = ptp.tile([PT, 512], MM_DT, tag="pt", bufs=6)
                        nc.scalar.activation(
                            out=pt_t[:, 0:w_t], in_=qt[:, 0:w_t],
                            func=AX.Exp, bias=zbias, scale=1.0,
                        )
                        if T == 0:
                            nc.vector.scalar_tensor_tensor(
                                out=pt_t[:, 0:PT], in0=pt_t[:, 0:PT], scalar=1.0,
                                in1=tri_sb, op0=OP.mult, op1=OP.mult,
                            )
                        tile_pieces = [p for p in pvp
                                       if p[0] - i_lo - 512 * T >= 0
                                       and p[1] - i_lo - 512 * T <= w_t]
                        # pieces overlapping the masked diagonal block wait on
                        # the DVE mask multiply; issue the unmasked ones first
                        tile_pieces.sort(key=lambda p: p[0] - i_lo < PT)
                        for (a, b2, _is_new, _is_diag) in tile_pieces:
                            la = a - i_lo - 512 * T
                            lb = b2 - i_lo - 512 * T
                            bank = a // 512
                            st_f = bank not in bank_started
                            bank_started.add(bank)
                            nc.tensor.matmul(
                                outacc[0:D + 1, a:b2],
                                v_all[:, tj, s, :],
                                pt_t[:, la:lb],
                                start=st_f, stop=(bank_last[bank] == (tj, a)),
                                skip_group_check=True,
                            )
                    if s == 0:
                        for fn, args in pending[:2]:
                            fn(*args)
                        del pending[:2]
                    if final and tj % 4 == 3:
                        # query tiles of 512-col group g are final after key
                        # tile 4g+3: stream their normalize, then the output
                        # projection + y store of the PREVIOUS group (its
                        # normalize chain has had a full group of tensor work
                        # to complete behind).
                        gw = 512
                        gsl = slice((tj // 4) * 512, (tj // 4) * 512 + 512)
                        oatg = nrm.tile([D + 1, gw], F32, tag=f"oatg{gw}", bufs=2)
                        nc.vector.tensor_scalar(
                            out=oatg, in0=outacc[0:D + 1, gsl], scalar1=1.0,
                            scalar2=0.0, op0=OP.mult, op1=OP.add)
                        rg = nrm.tile([D, gw // D], F32, tag=f"rg{gw}", bufs=2)
                        nc.gpsimd.dma_start(
                            out=rg,
                            in_=oatg[D:D + 1, :].rearrange("a (p t) -> a p t", p=D))
                        rrg = nrm.tile([D, gw // D], F32, tag=f"rrg{gw}", bufs=2)
                        nc.vector.reciprocal(out=rrg, in_=rg)
                        scrg = drp.tile([1, gw], F32, tag=f"scrg{gw}")
                        nc.gpsimd.dma_start(
                            out=scrg[0:1, :].rearrange("a (p t) -> a p t", p=D),
                            in_=rrg)
                        rbcg = nrm.tile([D, gw], F32, tag=f"rbcg{gw}", bufs=2)
                        nc.sync.dma_start(
                            out=rbcg, in_=scrg[0:1, :].to_broadcast([D, gw]))
                        ostg = nrm.tile([D, gw], MM_DT, tag=f"ostg{gw}", bufs=2)
                        nc.vector.scalar_tensor_tensor(
                            out=ostg, in0=oatg[0:D, :], scalar=1.0,
                            in1=rbcg, op0=OP.mult, op1=OP.mult,
                        )
                        nc.gpsimd.dma_start(out=onorm[D:PT, 1, gsl], in_=ostg)
                        if tj > 3:
                            for mt in range(tj - 7, tj - 3):
                                out_proj(mt)
                if final:
                    for mt in range(NT - 4, NT):
                        out_proj(mt)
                    continue
                # evict accumulator to SBUF, then normalize: reshape the
                # ones-row to 64 lanes (contiguous 128B runs), reciprocal,
                # bounce through DRAM to broadcast across partitions.
                oat = nrm.tile([D + 1, S], F32, tag="oat", bufs=2)
                nc.vector.tensor_scalar(out=oat, in0=outacc[0:D + 1, :],
                                        scalar1=1.0, scalar2=0.0,
                                        op0=OP.mult, op1=OP.add)
                r64 = nrm.tile([D, S // D], F32, tag="r64", bufs=2)
                nc.gpsimd.dma_start(
                    out=r64, in_=oat[D:D + 1, :].rearrange("a (p t) -> a p t", p=D))
                rr = nrm.tile([D, S // D], F32, tag="rr", bufs=2)
                nc.vector.reciprocal(out=rr, in_=r64)
                scr = drp.tile([1, S], F32, tag="scr")
                nc.gpsimd.dma_start(
                    out=scr[0:1, :].rearrange("a (p t) -> a p t", p=D), in_=rr)
                rbc = nrm.tile([D, S], F32, tag="rbc", bufs=2)
                nc.sync.dma_start(out=rbc, in_=scr[0:1, :].to_broadcast([D, S]))
                if not odd:
                    nc.vector.scalar_tensor_tensor(
                        out=onorm[0:D, ct, :], in0=oat[0:D, :], scalar=1.0,
                        in1=rbc, op0=OP.mult, op1=OP.mult,
                    )
                else:
                    ost = nrm.tile([D, S], MM_DT, tag="ost", bufs=2)
                    nc.vector.scalar_tensor_tensor(
                        out=ost, in0=oat[0:D, :], scalar=1.0,
                        in1=rbc, op0=OP.mult, op1=OP.mult,
                    )
                    # partition shift 0:64 -> 64:128 rides on SBUF->SBUF DMAs
                    for ch4 in range(4):
                        c5 = slice(ch4 * 512, ch4 * 512 + 512)
                        q_eng = (nc.sync, nc.gpsimd, nc.gpsimd, nc.sync)[ch4]
                        q_eng.dma_start(out=onorm[D:PT, ct, c5], in_=ost[:, c5])

    nc.finalize()
    return nc


def _prep_core_inputs(c, x, Wq, bq, Wk, bk, Wv, Wo):
    b, g = c // 4, c % 4
    heads = [g + 4 * k for k in range(SLOTS)]
    cidx = np.concatenate([np.arange(h * D, (h + 1) * D) for h in heads])
    slopes = _alibi_slopes(H)
    j = np.arange(S, dtype=np.float64)
    augm = np.empty((SLOTS, 12, S), dtype=np.float32)
    for k, h in enumerate(heads):
        sj = (slopes[h] * j).astype(np.float32)
        si = (-slopes[h] * j).astype(np.float32)
        augm[k, 0:3] = np.stack(_split3(sj))
        augm[k, 3:9] = 1.0
        augm[k, 9:12] = np.stack(_split3(si))
    tri = np.where(
        np.arange(PT)[:, None] <= np.arange(PT)[None, :], 1.0, 0.0
    ).astype(np.float32)
    wire = mybir.dt.np(MM_DT)
    return {
        "xT": np.ascontiguousarray(np.asarray(x[b], dtype=np.float32).T).astype(wire),
        "wqT": np.ascontiguousarray(np.asarray(Wq, np.float32)[cidx, :].T).astype(wire),
        "wkT": np.ascontiguousarray(np.asarray(Wk, np.float32)[cidx, :].T).astype(wire),
        "wvT": np.ascontiguousarray(np.asarray(Wv, np.float32)[cidx, :].T).astype(wire),
        "woT": np.ascontiguousarray(np.asarray(Wo, np.float32)[:, cidx].T).astype(wire),
        "bqk": np.stack([np.asarray(bq, np.float32)[cidx] / 8.0,
                         np.asarray(bk, np.float32)[cidx]]).astype(np.float32),
        "aug": augm.astype(wire),
        "trineg": tri.astype(wire),
    }


def kernel(x, Wq, bq, Wk, bk, Wv, bv, Wo, bo):
    if "nc" not in _CACHE:
        _CACHE["nc"] = _build_program()
    nc = _CACHE["nc"]

    in_maps = [_prep_core_inputs(c, x, Wq, bq, Wk, bk, Wv, Wo) for c in range(NCORES)]
    trace = os.environ.get("BASS_KERNEL_TRACE") == "1"
    res = run_bass_kernel_spmd(nc, in_maps, list(range(NCORES)), trace=trace)
    _CACHE["last_exec_time_ns"] = res.exec_time_ns

    bo_eff = (np.asarray(bo, np.float64)
              + np.asarray(Wo, np.float64) @ np.asarray(bv, np.float64))
    out = np.empty((B, S, E), dtype=np.float32)
    for b in range(B):
        acc = np.zeros((S, E), dtype=np.float64)
        for g in range(4):
            acc += np.asarray(res.results[b * 4 + g]["y"], dtype=np.float64)
        out[b] = (acc + bo_eff).astype(np.float32)
    return out
